# revision 27
# baseline (speedup 1.0000x reference)
"""Trainium2 Bass kernel for nn_AttentionGCNLayer (B=2, N=4096, D=256, H=2, ITERS=2).

Sharding: 8 cores = (b in 2) x (h in 2) x (row-half in 2). Each core handles one
(batch, head) pair and one half (2048) of the attention rows, with a pairwise
AllGather of the updated node features between the two GCN iterations.

Layout choices (per core):
  - x is kept transposed (x^T, [D=2x128 partitions, N free]) in *local* row
    order: columns [0:2048) are this core's rows, [2048:4096) the partner's.
    The aggregation sum over neighbors is permutation invariant, so local
    ordering is consistent as long as k/h/E all use the same order (they do).
  - scores are computed transposed (E^T = exp(q k^T / sqrt(dk))^T with
    [neighbor n on partitions, attention rows on free]) so that the
    neighbor-aggregation matmul consumes E^T directly, with no transposes.
  - softmax normalizer: rows of exp(scores) are summed with a ones-vector
    matmul on the PE; attn @ h / degs == (E @ h) * (1/R) with R = rowsum(E)
    (degs == 1 up to fp rounding, matching the reference within fp32 noise).
    scores are in [-1, 1], so exp needs no max-subtraction.
  - big matmuls run in bf16 (inputs) with fp32 PSUM accumulation.
"""

import sys

if "/opt/trn_rl_repo" not in sys.path:
    sys.path.insert(0, "/opt/trn_rl_repo")

import numpy as np

B, N, D, H, ITERS = 2, 4096, 256, 2, 2
DK = D // H                      # 128
RH = N // 2                      # 2048 rows per core
NCH = N // 128                   # 32 neighbor chunks
HCH = NCH // 2                   # 16 chunks per half
RT = 512                         # row tile (one PSUM bank of fp32)
NRT = RH // RT                   # 4 row tiles per core
SCALE = 1.0 / float(np.sqrt(np.float32(DK)))

_CACHE = {}


def _seq_engines(mybir):
    return {
        mybir.EngineType.PE,
        mybir.EngineType.Activation,
        mybir.EngineType.Pool,
        mybir.EngineType.DVE,
        mybir.EngineType.SP,
    }


def _split_excess_waits(nc, mybir, max_waits=1):
    """This container's walrus accepts at most one sync-wait per engine
    instruction; hoist extra waits onto preceding NoOps on the same engine."""
    seq = _seq_engines(mybir)
    n_new = 0
    for f in nc.m.functions:
        for blk in f.blocks:
            if not any(
                inst.sync_info is not None
                and inst.sync_info.on_wait
                and len(inst.sync_info.on_wait) > max_waits
                and inst.engine in seq
                for inst in blk.instructions
            ):
                continue
            out = []
            for inst in blk.instructions:
                si = inst.sync_info
                if (
                    si is not None
                    and si.on_wait
                    and len(si.on_wait) > max_waits
                    and inst.engine in seq
                ):
                    waits = list(si.on_wait)
                    keep, extra = waits[:max_waits], waits[max_waits:]
                    while extra:
                        chunk, extra = extra[:max_waits], extra[max_waits:]
                        out.append(
                            mybir.InstNoOp(
                                name=f"{inst.name}-ws{n_new}",
                                sync_info=mybir.SyncInfo(on_wait=chunk, on_update=[]),
                                bass_nofuse=True,
                                engine=inst.engine,
                            )
                        )
                        n_new += 1
                    inst.sync_info = mybir.SyncInfo(
                        on_wait=keep, on_update=list(si.on_update)
                    )
                out.append(inst)
            blk.instructions = out
    return n_new


def _build():
    import concourse.bass as bass
    import concourse.mybir as mybir
    import concourse.tile as tile
    from concourse.masks import make_identity

    f32 = mybir.dt.float32
    bf16 = mybir.dt.bfloat16
    fp8 = mybir.dt.float8e4
    AF = mybir.ActivationFunctionType

    nc = bass.Bass("TRN2", num_devices=8)

    nodes = nc.dram_tensor("nodes", [N, D], f32, kind="ExternalInput")
    wq = nc.dram_tensor("wq", [D, DK], f32, kind="ExternalInput")
    wk = nc.dram_tensor("wk", [D, DK], f32, kind="ExternalInput")
    wqb = nc.dram_tensor("wqb", [DK, 1], f32, kind="ExternalInput")
    wkb = nc.dram_tensor("wkb", [DK, 1], f32, kind="ExternalInput")
    gw = nc.dram_tensor("gw", [ITERS, D, D], f32, kind="ExternalInput")
    gb = nc.dram_tensor("gb", [ITERS, 2, 128, 1], f32, kind="ExternalInput")
    agg = nc.dram_tensor("agg", [D, D], f32, kind="ExternalInput")
    m0d = nc.dram_tensor("m0", [128, 1], f32, kind="ExternalInput")
    m1d = nc.dram_tensor("m1", [128, 1], f32, kind="ExternalInput")
    part = nc.dram_tensor("part", [RH, D], f32, kind="ExternalOutput")

    with tile.TileContext(nc) as tc:
        from contextlib import ExitStack

        with ExitStack() as ctx:
            const = ctx.enter_context(tc.tile_pool(name="const", bufs=1))

            ident = const.tile([128, 128], f32, name="ident")
            make_identity(nc, ident)
            ones_col = const.tile([128, 2, 16], fp8, name="ones_col")
            nc.vector.memset(ones_col, 1.0)
            ones_row = const.tile([1, 128], f32, name="ones_row")
            nc.vector.memset(ones_row, 1.0)

            # persistent state
            # x^T in bf16, split by feature chunk (dc) and row half (a=mine, b=partner)
            xT = [
                [
                    const.tile([128, RH], bf16, name=f"xT{dc}{hf}")
                    for hf in range(2)
                ]
                for dc in range(2)
            ]
            eP = [const.tile([128, 2, RH], fp8, name=f"eP{i}") for i in range(NCH // 2)]
            rinvB = const.tile([128, RH], f32, name="rinvB")

            # small weights/biases
            wq_s = const.tile([128, 2, DK], bf16, name="wq_s")
            wk_s = const.tile([128, 2, DK], bf16, name="wk_s")
            gw_s = const.tile([128, ITERS, 2, D], bf16, name="gw_s")
            agg_s = const.tile([128, 2, D], bf16, name="agg_s")
            wqb_s = const.tile([128, 1], f32, name="wqb_s")
            wkb_s = const.tile([128, 1], f32, name="wkb_s")
            gb_s = const.tile([128, ITERS, 2, 1], f32, name="gb_s")
            m0_s = const.tile([128, 1], f32, name="m0_s")
            m1_s = const.tile([128, 1], f32, name="m1_s")

            nc.gpsimd.dma_start(out=wqb_s, in_=wqb[:, :])
            nc.gpsimd.dma_start(out=wkb_s, in_=wkb[:, :])
            nc.gpsimd.dma_start(out=m0_s, in_=m0d[:, :])
            nc.gpsimd.dma_start(out=m1_s, in_=m1d[:, :])
            for i in range(ITERS):
                for dc in range(2):
                    nc.gpsimd.dma_start(out=gb_s[:, i, dc, :], in_=gb[i, dc, :, :])

            # pools used from P0 onward
            kq = ctx.enter_context(tc.tile_pool(name="kq", bufs=1))
            kT = kq.tile([128, N], bf16, name="kT")
            qT = kq.tile([128, RH], bf16, name="qT")
            ps_h = ctx.enter_context(tc.tile_pool(name="ps_h", bufs=1, space="PSUM"))

            def kq_gen(ws, bias_s, dst, hf, col):
                ps = ps_tr.tile([128, RT], f32, name="psk", tag="psk", bufs=2)
                for dc in range(2):
                    nc.tensor.matmul(
                        ps,
                        ws[:, dc, :],
                        xT[dc][hf][:, col : col + RT],
                        start=(dc == 0),
                        stop=(dc == 1),
                    )
                dcol = hf * RH + col
                nc.vector.tensor_scalar_add(
                    out=dst[:, dcol : dcol + RT], in0=ps, scalar1=bias_s
                )

            # ---- P0: stage + cast weights, transpose nodes into x^T ----
            with tc.tile_pool(name="stg", bufs=4) as stg, tc.tile_pool(
                name="ps_tr", bufs=2, space="PSUM"
            ) as ps_tr:
                for dc in range(2):
                    ws = stg.tile([128, DK], f32, name="wstg", tag="wstg")
                    nc.gpsimd.dma_start(out=ws, in_=wq[dc * 128 : (dc + 1) * 128, :])
                    nc.vector.tensor_copy(out=wq_s[:, dc, :], in_=ws)
                    ws2 = stg.tile([128, DK], f32, name="wstg2", tag="wstg")
                    nc.gpsimd.dma_start(out=ws2, in_=wk[dc * 128 : (dc + 1) * 128, :])
                    nc.vector.tensor_copy(out=wk_s[:, dc, :], in_=ws2)
                for i in range(ITERS):
                    for dc in range(2):
                        ws = stg.tile([128, D], f32, name="gstg", tag="gstg")
                        nc.gpsimd.dma_start(
                            out=ws, in_=gw[i, dc * 128 : (dc + 1) * 128, :]
                        )
                        nc.vector.tensor_copy(out=gw_s[:, i, dc, :], in_=ws)
                for dc in range(2):
                    ws = stg.tile([128, D], f32, name="astg", tag="gstg")
                    nc.gpsimd.dma_start(out=ws, in_=agg[dc * 128 : (dc + 1) * 128, :])
                    nc.vector.tensor_copy(out=agg_s[:, dc, :], in_=ws)

                # nodes -> x^T (bf16): four contiguous 512KB DMAs. Partition
                # p of group g holds node rows g*512 + 4p..4p+3, so the local
                # column order is the permutation g*512 + c*128 + p  <->  node
                # row g*512 + 4p + c (undone on the host for the output).
                for g in range(8):
                    st = stg.tile([128, 4, D], f32, name="nstg", tag="nstg", bufs=3)
                    nc.sync.dma_start(
                        out=st,
                        in_=nodes[g * 512 : (g + 1) * 512, :].rearrange(
                            "(p c) d -> p c d", p=128
                        ),
                    )
                    hf = 0 if g < 4 else 1
                    for c in range(4):
                        col = (g % 4) * 512 + c * 128
                        for dc in range(2):
                            pt = ps_tr.tile([128, 128], f32, name="ptr", tag="ptr", bufs=4)
                            nc.tensor.transpose(
                                pt, st[:, c, dc * 128 : (dc + 1) * 128], ident
                            )
                            nc.vector.tensor_copy(
                                out=xT[dc][hf][:, col : col + 128], in_=pt
                            )
                    kq_gen(wk_s, wkb_s, kT, hf, (g % 4) * 512)
                    if hf == 0:
                        kq_gen(wq_s, wqb_s, qT, 0, (g % 4) * 512)

            # ---- P1 + GCN, rowtile-pipelined ----
            p1ctx = ExitStack()
            ps_sc = p1ctx.enter_context(tc.tile_pool(name="ps_sc", bufs=2, space="PSUM"))
            ps_u = p1ctx.enter_context(tc.tile_pool(name="ps_u", bufs=2, space="PSUM"))
            hP = [const.tile([128, 2, D], fp8, name=f"hP{i}") for i in range(NCH // 2)]
            racc = ctx.enter_context(tc.tile_pool(name="racc", bufs=2))
            upd = ctx.enter_context(tc.tile_pool(name="upd", bufs=3))
            dram = ctx.enter_context(tc.tile_pool(name="dram", bufs=1, space="DRAM"))
            HR = RH // 2
            cc_in = [dram.tile([2 * 128, HR], bf16, name=f"cc_in{g}") for g in range(2)]
            cc_out = [dram.tile([4 * 128, HR], bf16, name=f"cc_out{g}") for g in range(2)]

            def scores_exp(mt):  # mega rowtile of 1024
                for ncx in range(NCH):
                    ps = ps_sc.tile([128, 2 * RT], f32, name="pss", tag="pss")
                    for j in range(2):
                        nc.tensor.matmul(
                            ps[:, j * RT : (j + 1) * RT],
                            kT[:, ncx * 128 : (ncx + 1) * 128],
                            qT[:, (2 * mt + j) * RT : (2 * mt + j + 1) * RT],
                            start=True,
                            stop=True,
                        )
                    nc.scalar.activation(
                        out=eP[ncx // 2][
                            :, ncx % 2, 2 * mt * RT : (2 * mt + 2) * RT
                        ],
                        in_=ps,
                        func=AF.Exp,
                        scale=SCALE,
                    )

            def r_reduce(rt):
                # R = sum_n E via DoubleRow ones-matmuls, then broadcast +
                # reciprocal
                ps_row = ps_sc.tile([1, RT], f32, name="psrow", tag="psrow", bufs=1)
                for cp in range(NCH // 2):
                    nc.tensor.matmul(
                        ps_row,
                        ones_col[:, :, 0:1],
                        eP[cp][:, :, rt * RT : (rt + 1) * RT],
                        start=(cp == 0),
                        stop=(cp == NCH // 2 - 1),
                        perf_mode=mybir.MatmulPerfMode.DoubleRow,
                    )
                rrow = racc.tile([1, RT], f32, name="rrow", tag="rrow")
                nc.vector.tensor_copy(out=rrow, in_=ps_row)
                ps_b = ps_sc.tile([128, RT], f32, name="psb", tag="psrow", bufs=1)
                nc.tensor.matmul(ps_b, ones_row, rrow, start=True, stop=True)
                nc.vector.reciprocal(
                    out=rinvB[:, rt * RT : (rt + 1) * RT], in_=ps_b
                )

            def h_gen(it, half, rng=None):
                if rng is None:
                    rng = range(HCH) if half == 0 else range(HCH, NCH)
                for ncx in rng:
                    hf, col = (
                        (0, ncx * 128) if ncx < HCH else (1, (ncx - HCH) * 128)
                    )
                    ps = ps_h.tile([128, D], f32, name="psh", tag="psh")
                    for dc in range(2):
                        nc.tensor.matmul(
                            ps,
                            xT[dc][hf][:, col : col + 128],
                            gw_s[:, it, dc, :],
                            start=(dc == 0),
                            stop=(dc == 1),
                        )
                    nc.scalar.activation(
                        out=hP[ncx // 2][:, ncx % 2, :], in_=ps, func=AF.Copy
                    )

            def agg_mms(pool, it, rt, cps, pu=None):
                if pu is None:
                    pu = [
                        pool.tile([128, RT], f32, name=f"pu{dc}", tag="pu")
                        for dc in range(2)
                    ]
                for cp in cps:
                    for dc in range(2):
                        nc.tensor.matmul(
                            pu[dc],
                            hP[cp][:, :, dc * 128 : (dc + 1) * 128],
                            eP[cp][:, :, rt * RT : (rt + 1) * RT],
                            start=(cp == 0),
                            stop=(cp == NCH // 2 - 1),
                            perf_mode=mybir.MatmulPerfMode.DoubleRow,
                        )
                return pu

            def agg_update(it, rt, pool=None, pu=None):
                if pu is None:
                    pu = agg_mms(pool if pool is not None else ps_u, it, rt, range(NCH // 2))
                else:
                    agg_mms(None, it, rt, range(NCH // 4, NCH // 2), pu=pu)
                for dc in range(2):
                    t = upd.tile([128, RT], f32, name="updt", tag="updt")
                    nc.vector.tensor_mul(
                        t, pu[dc], rinvB[:, rt * RT : (rt + 1) * RT]
                    )
                    nc.scalar.activation(
                        out=t,
                        in_=t,
                        func=AF.Relu,
                        bias=gb_s[:, it, dc, :],
                        scale=1.0,
                    )
                    nc.vector.tensor_add(
                        out=xT[dc][0][:, rt * RT : (rt + 1) * RT],
                        in0=xT[dc][0][:, rt * RT : (rt + 1) * RT],
                        in1=t,
                    )
                if it == 0:
                    for dc in range(2):
                        nc.sync.dma_start(
                            out=cc_in[rt // 2][
                                dc * 128 : (dc + 1) * 128,
                                (rt % 2) * RT : (rt % 2 + 1) * RT,
                            ],
                            in_=xT[dc][0][:, rt * RT : (rt + 1) * RT],
                        )

            # pipeline: h1 in the idle head, then scores/exp by 1024-wide
            # mega-rowtiles with [R, agg1, update1] per 512-rowtile behind;
            # the x1 exchange fires in two halves so the first AllGather
            # overlaps the second mega-rowtile of attention compute
            def fire_cc(g):
                nc.gpsimd.collective_compute(
                    "AllGather",
                    mybir.AluOpType.bypass,
                    replica_groups=[[0, 1], [2, 3], [4, 5], [6, 7]],
                    ins=[cc_in[g][:, :].opt()],
                    outs=[cc_out[g][:, :].opt()],
                )

            cct = ctx.enter_context(tc.tile_pool(name="cct", bufs=4))

            def combine(g):
                # partner rows g*1024..g*1024+1024 -> local cols 2048+g*1024..
                for ct in range(2):
                    for dc in range(2):
                        t0 = cct.tile([128, RT], bf16, name="t0", tag="cct")
                        t1 = cct.tile([128, RT], bf16, name="t1", tag="cct")
                        nc.sync.dma_start(
                            out=t0,
                            in_=cc_out[g][
                                dc * 128 : (dc + 1) * 128,
                                ct * RT : (ct + 1) * RT,
                            ],
                        )
                        nc.sync.dma_start(
                            out=t1,
                            in_=cc_out[g][
                                256 + dc * 128 : 256 + (dc + 1) * 128,
                                ct * RT : (ct + 1) * RT,
                            ],
                        )
                        nc.vector.tensor_scalar_mul(t0, t0, m1_s)
                        nc.vector.tensor_scalar_mul(t1, t1, m0_s)
                        nc.vector.tensor_add(
                            out=xT[dc][1][
                                :,
                                g * 2 * RT + ct * RT : g * 2 * RT + (ct + 1) * RT,
                            ],
                            in0=t0,
                            in1=t1,
                        )

            scores_exp(0)
            h_gen(0, 0)
            h_gen(0, 1)
            for mt in range(NRT // 2):
                if mt + 1 < NRT // 2:
                    scores_exp(mt + 1)
                r_reduce(2 * mt)
                r_reduce(2 * mt + 1)
                agg_update(0, 2 * mt)
                agg_update(0, 2 * mt + 1)
                fire_cc(mt)
                if mt == 0:
                    combine(0)

            p1ctx.close()
            h_gen(1, 0)
            combine(1)
            h_gen(1, 1, range(HCH, HCH + HCH // 2))
            h_gen(1, 1, range(HCH + HCH // 2, NCH))
            ost = ctx.enter_context(tc.tile_pool(name="ost", bufs=3))

            def out_chunk(rc):
                ps = ps_h.tile([128, D], f32, name="pso", tag="psh")
                for dc in range(2):
                    nc.tensor.matmul(
                        ps,
                        xT[dc][0][:, rc * 128 : (rc + 1) * 128],
                        agg_s[:, dc, :],
                        start=(dc == 0),
                        stop=(dc == 1),
                    )
                ot = ost.tile([128, D], f32, name="ot", tag="ot")
                nc.scalar.activation(out=ot, in_=ps, func=AF.Copy)
                nc.sync.dma_start(out=part[rc * 128 : (rc + 1) * 128, :], in_=ot)

            with tc.tile_pool(name="ps_u2", bufs=6, space="PSUM") as ps_u2:
                pus = [agg_mms(ps_u2, 1, rt, range(NCH // 4)) for rt in range(3)]
                for rt in range(NRT):
                    if rt < 3:
                        agg_update(1, rt, pu=pus[rt])
                    else:
                        agg_update(1, rt, pool=ps_u2)
                    for rc in range(4 * rt, 4 * rt + 4):
                        out_chunk(rc)

    _split_excess_waits(nc, mybir)
    return nc


def _get_nc():
    if "nc" not in _CACHE:
        _CACHE["nc"] = _build()
    return _CACHE["nc"]


def _in_maps(inputs):
    ne = np.asarray(inputs["nodes_embed"], dtype=np.float32)
    wq_w = np.asarray(inputs["WQ_w"], dtype=np.float32)
    wq_b = np.asarray(inputs["WQ_b"], dtype=np.float32)
    wk_w = np.asarray(inputs["WK_w"], dtype=np.float32)
    wk_b = np.asarray(inputs["WK_b"], dtype=np.float32)
    gcn_w = np.asarray(inputs["gcn_W"], dtype=np.float32)
    gcn_b = np.asarray(inputs["gcn_b"], dtype=np.float32)
    agg_w = np.asarray(inputs["agg_W"], dtype=np.float32)

    gb = np.ascontiguousarray(gcn_b.reshape(ITERS, 2, 128, 1))
    maps = []
    for c in range(8):
        b, h, rh = c // 4, (c // 2) % 2, c % 2
        if rh == 0:
            nodes = ne[b]
        else:
            nodes = np.concatenate([ne[b, RH:], ne[b, :RH]], axis=0)
        m0 = np.full((128, 1), 1.0 if rh == 0 else 0.0, np.float32)
        m1 = np.full((128, 1), 0.0 if rh == 0 else 1.0, np.float32)
        maps.append(
            {
                "nodes": np.ascontiguousarray(nodes),
                "wq": np.ascontiguousarray(wq_w[:, h * DK : (h + 1) * DK]),
                "wk": np.ascontiguousarray(wk_w[:, h * DK : (h + 1) * DK]),
                "wqb": np.ascontiguousarray(
                    wq_b[h * DK : (h + 1) * DK].reshape(DK, 1)
                ),
                "wkb": np.ascontiguousarray(
                    wk_b[h * DK : (h + 1) * DK].reshape(DK, 1)
                ),
                "gw": gcn_w,
                "gb": gb,
                "agg": np.ascontiguousarray(agg_w[h * D : (h + 1) * D, :]),
                "m0": m0,
                "m1": m1,
            }
        )
    return maps


def kernel(trace=False, tmpdir=None, **inputs):
    from concourse.bass_utils import run_bass_kernel_spmd

    nc = _get_nc()
    maps = _in_maps(inputs)
    kw = {}
    if trace:
        kw = dict(trace=True, tmpdir=tmpdir)
    res = run_bass_kernel_spmd(nc, maps, core_ids=list(range(8)), **kw)

    agg_b = np.asarray(inputs["agg_b"], dtype=np.float32)
    out = np.zeros((B, N, D), np.float32)
    for b in range(B):
        for rh in range(2):
            rows = slice(rh * RH, (rh + 1) * RH)
            p = (
                res.results[4 * b + 0 * 2 + rh]["part"]
                + res.results[4 * b + 1 * 2 + rh]["part"]
                + agg_b
            )
            # undo the load permutation: out row g*512 + 4p + c was written to
            # part row g*512 + c*128 + p
            p = (
                p.reshape(4, 4, 128, D).transpose(0, 2, 1, 3).reshape(RH, D)
            )
            out[b, rows, :] = p
    if trace:
        return out, res
    return out
